# revision 1
# baseline (speedup 1.0000x reference)
"""Trainium2 Bass kernel for the Castro2025 RL model (T=1000, B=8192, P=1024, A=4).

Sharding: batch (sessions) split across 8 NeuronCores; the sequential scan
over T runs on-device per core; per-participant params replicated.
Self-contained: includes the harness patches it needs.
"""
import sys
import types
import numpy as np
import concourse.bass as bass
import concourse.bacc as bacc
import concourse.mybir as mybir
import concourse.tile as tile
from concourse.vector_clock import ScopedClock

# ---- harness patch: this walrus build rejects any instruction carrying more
# than one semaphore wait; split extras onto single-wait NOPs beforehand. ----
MAX_WAITS = 1
if not getattr(tile, "_waitsplit_patched", False):
    _orig_postorder = tile.postorder_instruction_blocks

    def _split_waits_postorder(ordered, start_bb, postordered):
        out = _orig_postorder(ordered, start_bb, postordered)
        for bb_name, insts in postordered.items():
            new_list = []
            for inst in insts:
                si = inst.sync_info
                if si is not None and si.on_wait and len(si.on_wait) > MAX_WAITS \
                        and inst.engine != mybir.EngineType.Unassigned \
                        and not isinstance(inst, tile.BassTileLoopBlock):
                    waits = list(si.on_wait)
                    keep = waits[-MAX_WAITS:]
                    extra = waits[:-MAX_WAITS]
                    for wi, w in enumerate(extra):
                        nop = mybir.InstNoOp(
                            name=f"I-waitsplit-{id(inst)}-{len(new_list)}-{wi}",
                            engine=inst.engine,
                            sync_info=mybir.SyncInfo(on_wait=[w], on_update=[]),
                        )
                        new_list.append(nop)
                    si.on_wait = keep
                new_list.append(inst)
            insts[:] = new_list
        return out

    tile.postorder_instruction_blocks = _split_waits_postorder

    def _patched_drain_and_barrier(self, tick_clock, wait_clock):
        probe = mybir.InstNoOp(name=f"I-{self.nc.next_id()}", engine=mybir.EngineType.SP)
        wait_clock.add_sem_waits(probe, ScopedClock({None: tick_clock.global_clock}))
        waits = list(probe.sync_info.on_wait) if probe.sync_info and probe.sync_info.on_wait else []
        for w in waits:
            nop = self.nc.sync.nop(nofuse=True, hint="drain_split_wait")
            if nop.ins.sync_info is None:
                nop.ins.sync_info = mybir.SyncInfo(on_wait=[w], on_update=[])
            else:
                nop.ins.sync_info.on_wait = [w]
        self.nc.sync.drain()
        self.nc.all_engine_barrier()
        popped = self.nc._tile_sem_poison_stack.pop()
        assert popped is self._sem_poison
        self.nc.clear_and_free_semaphores(list(self.sems.allocated().values()))
        self.nc.all_engine_barrier()

    tile.TileContext._drain_and_barrier = _patched_drain_and_barrier
    tile._waitsplit_patched = True

from concourse.bass_utils import run_bass_kernel_spmd  # noqa: E402

F32 = mybir.dt.float32
I32 = mybir.dt.int32
U8 = mybir.dt.uint8
AX = mybir.AxisListType
OP = mybir.AluOpType
AF = mybir.ActivationFunctionType

T, B, P, A = 1000, 8192, 1024, 4
NCORE = 8
BL = B // NCORE          # 1024 sessions per core
G = BL // 128            # 8 groups
ER_D = 1.0 - 1e-3
LN_ER_D = float(np.log(ER_D))
TCH = 50                 # chunk length
NCH = T // TCH


def bcA(ap, n=A):
    """[128, G, Tc] -> [128, G, n, Tc] broadcast on a new axis 2."""
    s = list(ap.shape)
    return ap.unsqueeze(2).broadcast_to([s[0], s[1], n] + s[2:])


def bcT(ap, n):
    """[128, G] or [128,G,A] -> broadcast with new trailing axis of n."""
    s = list(ap.shape)
    return ap.unsqueeze(len(s)).broadcast_to(s + [n])


def build_nc():
    nc = bacc.Bacc()
    rew_d = nc.declare_dram_parameter("rewards", [BL, T], F32, isOutput=False)
    cho_d = nc.declare_dram_parameter("choices", [BL, T], I32, isOutput=False)
    par_d = nc.declare_dram_parameter("paramsT", [P, 13], F32, isOutput=False)
    pid_d = nc.declare_dram_parameter("pids", [128, G], I32, isOutput=False)
    out_d = nc.declare_dram_parameter("out", [128, G, A, T], F32, isOutput=True)

    with tile.TileContext(nc) as tc:
        import contextlib
        with contextlib.ExitStack() as ctx:
            _build(ctx, tc, nc, rew_d, cho_d, par_d, pid_d, out_d)
    nc.compile()
    return nc


def _build(ctx, tc, nc, rew_d, cho_d, par_d, pid_d, out_d):
    pp = ctx.enter_context(tc.tile_pool(name="persist", bufs=1))
    wp = ctx.enter_context(tc.tile_pool(name="work", bufs=1))
    wp2 = ctx.enter_context(tc.tile_pool(name="work2", bufs=2))

    # ---------- phase 0: params ----------
    pids = pp.tile([128, G], I32)
    nc.sync.dma_start(pids[:], pid_d[:])
    praw = pp.tile([128, G, 13], F32)
    for g in range(G):
        nc.gpsimd.indirect_dma_start(
            out=praw[:, g, :], out_offset=None, in_=par_d[:],
            in_offset=bass.IndirectOffsetOnAxis(ap=pids[:, g:g + 1], axis=0))
    nc.vector.tensor_scalar(praw[:], praw[:], -5.0, 5.0, op0=OP.max, op1=OP.min)

    c_one = pp.tile([128, 1], F32)
    nc.vector.memset(c_one[:], 1.0)
    czero = pp.tile([128, 1], F32)
    nc.vector.memset(czero[:], 0.0)
    c_lnd = pp.tile([128, 1], F32)
    nc.vector.memset(c_lnd[:], LN_ER_D)

    def sp(dst, src):
        # softplus = ln(1+exp(x)); safe since |x| <= 5 after clip
        nc.scalar.activation(dst, src, AF.Exp)
        nc.scalar.activation(dst, dst, AF.Ln, bias=c_one[:])

    def sg(dst, src):
        nc.scalar.activation(dst, src, AF.Sigmoid)

    def clip(ap, lo, hi):
        nc.vector.tensor_scalar(ap, ap, float(lo), float(hi), op0=OP.max, op1=OP.min)

    pv = pp.tile([128, G, 16], F32)
    BETA_R, LAPSE, PRIOR, AER, DECAY, AB1, AB2, PERV, SW, GAM, TEMP, BETA_P, A1, L4, OML, PWSW = range(16)
    sp(pv[:, :, BETA_R], praw[:, :, 0]); clip(pv[:, :, BETA_R], 0.01, 20.0)
    sg(pv[:, :, LAPSE], praw[:, :, 1]); clip(pv[:, :, LAPSE], 0.01, 0.99)
    sp(pv[:, :, PRIOR], praw[:, :, 2]); clip(pv[:, :, PRIOR], 0.01, 0.99)
    sg(pv[:, :, AER], praw[:, :, 3]); clip(pv[:, :, AER], 0.01, 0.99)
    sg(pv[:, :, DECAY], praw[:, :, 4]); clip(pv[:, :, DECAY], 0.01, 0.99)
    nc.vector.tensor_copy(pv[:, :, AB1], praw[:, :, 5])
    nc.vector.tensor_copy(pv[:, :, AB2], praw[:, :, 6])
    sp(pv[:, :, PERV], praw[:, :, 7])
    nc.vector.tensor_copy(pv[:, :, SW], praw[:, :, 8])
    sp(pv[:, :, GAM], praw[:, :, 10])
    sp(pv[:, :, TEMP], praw[:, :, 11])
    nc.vector.tensor_scalar(pv[:, :, TEMP], pv[:, :, TEMP], 1e-6, None, op0=OP.add)
    clip(pv[:, :, TEMP], 1e-6, 100.0)
    sp(pv[:, :, BETA_P], praw[:, :, 12])
    rtmp = pp.tile([128, G], F32)
    nc.vector.reciprocal(rtmp[:], pv[:, :, TEMP])
    nc.vector.tensor_tensor(pv[:, :, A1], pv[:, :, BETA_R], rtmp[:], op=OP.mult)
    nc.vector.tensor_scalar(pv[:, :, L4], pv[:, :, LAPSE], 0.25, None, op0=OP.mult)
    nc.vector.tensor_scalar(pv[:, :, OML], pv[:, :, LAPSE], -1.0, 1.0, op0=OP.mult, op1=OP.add)
    nc.vector.tensor_tensor(pv[:, :, PWSW], pv[:, :, PERV], pv[:, :, SW], op=OP.subtract)

    # dpow[t] = 0.999^(t+1)
    dpow = pp.tile([128, T], F32)
    ii32 = pp.tile([128, T], I32)
    nc.gpsimd.iota(ii32[:], pattern=[[1, T]], base=0, channel_multiplier=0)
    nc.vector.tensor_copy(dpow[:], ii32[:])
    nc.scalar.activation(dpow[:], dpow[:], AF.Exp, bias=c_lnd[:], scale=c_lnd[:])

    # reset mask for packed scans
    rmGA = pp.tile([128, G, A, TCH], F32)
    nc.vector.memset(rmGA[:], 1.0)
    nc.vector.memset(rmGA[:, :, :, 0:1], 0.0)
    rmG = rmGA[:, :, 0, :]

    # persistent state (split into two independent half-chains)
    qA = pp.tile([128, G // 2, A], F32)
    qB = pp.tile([128, G // 2, A], F32)
    qF = pp.tile([128, G, A], F32)
    nc.vector.tensor_copy(qF[:], bcT(pv[:, :, PRIOR], A))
    cumc = pp.tile([128, G, A], F32)
    nc.vector.memset(cumc[:], 0.0)
    tslc = pp.tile([128, G], F32)
    nc.vector.memset(tslc[:], 0.0)

    # software-pipelined emission: prep(c) -> epilogue(c-1) -> loop(c)
    st = {}

    def prep(c):
        t0 = c * TCH
        d = {}
        cho = wp2.tile([128, G, TCH + 1], I32, tag="cho")
        if c == 0:
            nc.vector.memset(cho[:, :, 0:1], -1)
            nc.sync.dma_start(cho[:, :, 1:].opt(), cho_d[:, t0:t0 + TCH].rearrange("(g p) t -> p g t", p=128))
        else:
            nc.sync.dma_start(cho[:].opt(), cho_d[:, t0 - 1:t0 + TCH].rearrange("(g p) t -> p g t", p=128))
        rew = wp2.tile([128, G, TCH], F32, tag="rew")
        nc.sync.dma_start(rew[:], rew_d[:, t0:t0 + TCH].rearrange("(g p) t -> p g t", p=128))

        tgt = wp2.tile([128, G, TCH], F32, tag="tgt")
        nc.gpsimd.tensor_scalar(tgt[:], rew[:], -1.0, None, op0=OP.add)
        nc.gpsimd.tensor_tensor(tgt[:], tgt[:], bcT(pv[:, :, GAM], TCH), op=OP.mult)
        nc.gpsimd.tensor_tensor(tgt[:], tgt[:], rew[:], op=OP.add)

        ohu = wp2.tile([128, G, A, TCH + 1], U8, tag="ohu")
        for a in range(A):
            nc.vector.tensor_scalar(ohu[:, :, a, :], cho[:], float(a), None, op0=OP.is_equal)
        ohcF = wp2.tile([128, G, A, TCH], F32, tag="ohcF")
        nc.gpsimd.tensor_copy(ohcF[:], ohu[:, :, :, 1:])
        same = wp2.tile([128, G, TCH], F32, tag="same")
        nc.vector.tensor_tensor(same[:], cho[:, :, 1:], cho[:, :, 0:TCH], op=OP.is_equal)

        At = wp2.tile([128, G, TCH], F32, tag="At")
        Bt = wp2.tile([128, G, TCH], F32, tag="Bt")
        dpc = dpow[:, t0:t0 + TCH].unsqueeze(1).broadcast_to([128, G, TCH])
        nc.gpsimd.tensor_tensor(At[:], bcT(pv[:, :, AER], TCH), dpc, op=OP.mult)
        nc.gpsimd.tensor_scalar(At[:], At[:], -1.0, 1.0, op0=OP.mult, op1=OP.add)
        nc.gpsimd.tensor_tensor(At[:], At[:], bcT(pv[:, :, DECAY], TCH), op=OP.mult)
        nc.gpsimd.tensor_tensor(Bt[:], bcT(pv[:, :, DECAY], TCH), At[:], op=OP.subtract)
        nc.gpsimd.tensor_scalar(Bt[:], Bt[:], 0.25, None, op0=OP.mult)

        qin = wp2.tile([128, G, A], F32, tag="qin")
        nc.vector.tensor_copy(qin[:], qF[:])
        d.update(cho=cho, rew=rew, tgt=tgt, ohu=ohu, ohcF=ohcF, same=same, At=At, Bt=Bt, qin=qin)
        return d

    def loop(c, d):
        spb = wp2.tile([128, G, TCH], F32, tag="spb")
        d["spb"] = spb
        tgt, At, Bt, ohu = d["tgt"], d["At"], d["Bt"], d["ohu"]
        tA = wp.tile([128, G // 2], F32, tag="tA")
        tB = wp.tile([128, G // 2], F32, tag="tB")
        tF = wp.tile([128, G], F32, tag="tF")
        H = G // 2
        INTERLEAVE = 1

        def lstep(t, qh, tmp, gl, gh, W):
            tgt_b = tgt[:, gl:gh, t].unsqueeze(2).broadcast_to([128, W, A])
            nc.vector.copy_predicated(qh[:], ohu[:, gl:gh, :, t + 1], tgt_b)
            nc.vector.tensor_reduce(spb[:, gl:gh, t], qh[:], axis=AX.X, op=OP.add)
            nc.vector.tensor_tensor(tmp[:], spb[:, gl:gh, t], Bt[:, gl:gh, t], op=OP.mult)
            nc.vector.tensor_tensor(qh[:], qh[:], At[:, gl:gh, t].unsqueeze(2).broadcast_to([128, W, A]), op=OP.mult)
            nc.vector.tensor_tensor(qh[:], qh[:], tmp[:].unsqueeze(2).broadcast_to([128, W, A]), op=OP.add)

        if INTERLEAVE == 2:
            for t in range(TCH):
                lstep(t, qA, tA, 0, H, H)
                lstep(t, qB, tB, H, G, H)
        else:
            for t in range(TCH):
                lstep(t, qF, tF, 0, G, G)

    def epilogue(c, d):
        t0 = c * TCH
        tgt, At, Bt, ohu, ohcF = d["tgt"], d["At"], d["Bt"], d["ohu"], d["ohcF"]
        same, spb, qin, rew = d["same"], d["spb"], d["qin"], d["rew"]
        al = wp2.tile([128, G, A, TCH], F32, tag="al")
        AtbA = bcA(At[:])
        nc.gpsimd.tensor_tensor(al[:], ohcF[:], AtbA, op=OP.mult)        # A*oh
        m1 = wp2.tile([128, G, TCH], F32, tag="m1")
        nc.gpsimd.tensor_tensor(m1[:], At[:], tgt[:], op=OP.mult)
        be = wp2.tile([128, G, A, TCH], F32, tag="be")
        nc.gpsimd.tensor_tensor(be[:], ohcF[:], bcA(m1[:]), op=OP.mult)  # oh*(A*tgt)
        nc.gpsimd.tensor_tensor(al[:], AtbA, al[:], op=OP.subtract)      # A*(1-oh)
        bsp = wp2.tile([128, G, TCH], F32, tag="bsp")
        nc.gpsimd.tensor_tensor(bsp[:], Bt[:], spb[:], op=OP.mult)
        nc.gpsimd.tensor_tensor(be[:], be[:], bcA(bsp[:]), op=OP.add)
        a0 = wp2.tile([128, G, A], F32, tag="a0")
        nc.vector.tensor_scalar(a0[:], ohcF[:, :, :, 0], -1.0, 1.0, op0=OP.mult, op1=OP.add)
        nc.vector.tensor_tensor(a0[:], a0[:], At[:, :, 0].unsqueeze(2).broadcast_to([128, G, A]), op=OP.mult)
        nc.vector.tensor_tensor(a0[:], a0[:], qin[:], op=OP.mult)
        nc.vector.tensor_tensor(be[:, :, :, 0], be[:, :, :, 0], a0[:], op=OP.add)
        nc.vector.memset(al[:, :, :, 0:1], 0.0)
        qs = wp2.tile([128, G, A, TCH], F32, tag="qs")
        nc.vector.tensor_tensor_scan(
            qs[:].rearrange("p g a t -> p (g a t)"),
            al[:].rearrange("p g a t -> p (g a t)"),
            be[:].rearrange("p g a t -> p (g a t)"),
            0.0, op0=OP.mult, op1=OP.add)

        cum = al
        nc.vector.tensor_tensor_scan(
            cum[:].rearrange("p g a t -> p (g a t)"),
            rmGA[:].rearrange("p g a t -> p (g a t)"),
            ohcF[:].rearrange("p g a t -> p (g a t)"),
            0.0, op0=OP.mult, op1=OP.add)
        nc.gpsimd.tensor_tensor(cum[:], cum[:], bcT(cumc[:], TCH), op=OP.add)
        nc.gpsimd.tensor_copy(cumc[:], cum[:, :, :, TCH - 1])

        d0 = rew
        d1 = wp2.tile([128, G, TCH], F32, tag="d1")
        nc.gpsimd.tensor_copy(d1[:], same[:])
        nc.gpsimd.tensor_tensor(d0[:], same[:], rmG, op=OP.mult)
        nc.gpsimd.tensor_scalar(tslc[:], tslc[:], 1.0, None, op0=OP.add)
        nc.gpsimd.tensor_tensor(d1[:, :, 0], same[:, :, 0], tslc[:], op=OP.mult)
        tsl = wp2.tile([128, G, TCH], F32, tag="tsl")
        nc.vector.tensor_tensor_scan(
            tsl[:].rearrange("p g t -> p (g t)"),
            d0[:].rearrange("p g t -> p (g t)"),
            d1[:].rearrange("p g t -> p (g t)"),
            0.0, op0=OP.mult, op1=OP.add)
        nc.gpsimd.tensor_copy(tslc[:], tsl[:, :, TCH - 1])

        nc.scalar.activation(cum[:], cum[:], AF.Ln, bias=c_one[:])
        sm = wp2.tile([128, G, A, TCH], F32, tag="sm")
        nc.vector.tensor_tensor(sm[:], qs[:], bcA(bcT(pv[:, :, A1], TCH)), op=OP.mult)
        nc.gpsimd.tensor_tensor(cum[:], cum[:], bcA(bcT(pv[:, :, BETA_P], TCH)), op=OP.mult)
        nc.vector.tensor_tensor(sm[:], sm[:], cum[:], op=OP.add)
        mx = wp2.tile([128, G, TCH], F32, tag="mx")
        nc.vector.tensor_reduce(mx[:], sm[:].rearrange("p g a t -> p g t a"), axis=AX.X, op=OP.max)
        nc.gpsimd.tensor_tensor(sm[:], sm[:], bcA(mx[:]), op=OP.subtract)
        nc.scalar.activation(sm[:], sm[:], AF.Exp)
        nc.vector.tensor_reduce(mx[:], sm[:].rearrange("p g a t -> p g t a"), axis=AX.X, op=OP.add)
        nc.vector.reciprocal(mx[:], mx[:])
        nc.gpsimd.tensor_tensor(mx[:], mx[:], bcT(pv[:, :, OML], TCH), op=OP.mult)
        nc.vector.tensor_tensor(sm[:], sm[:], bcA(mx[:]), op=OP.mult)
        nc.gpsimd.tensor_tensor(sm[:], sm[:], bcA(bcT(pv[:, :, L4], TCH)), op=OP.add)
        nc.scalar.activation(sm[:], sm[:], AF.Ln)

        nc.scalar.activation(tsl[:], tsl[:], AF.Ln, bias=c_one[:])
        nc.gpsimd.tensor_tensor(same[:], same[:], bcT(pv[:, :, PWSW], TCH), op=OP.mult)
        nc.gpsimd.tensor_tensor(same[:], same[:], bcT(pv[:, :, SW], TCH), op=OP.add)
        nc.gpsimd.tensor_tensor(same[:], same[:], tsl[:], op=OP.add)        # inner
        ext = be
        nc.gpsimd.tensor_tensor(ext[:], ohcF[:], bcA(same[:]), op=OP.mult)   # oh*inner
        nc.gpsimd.tensor_tensor(sm[:], sm[:], ext[:], op=OP.add)
        nc.vector.memset(ext[:], 0.0)
        nc.vector.copy_predicated(ext[:], ohu[:, :, :, 0:TCH], bcA(bcT(pv[:, :, AB1], TCH)))
        ab2b2 = bcT(pv[:, :, AB2], TCH).unsqueeze(2).broadcast_to([128, G, 2, TCH])
        exr = qs
        nc.vector.memset(exr[:], 0.0)
        nc.vector.copy_predicated(exr[:, :, 0:2, :], ohu[:, :, 2:4, 1:], ab2b2)
        nc.vector.copy_predicated(exr[:, :, 2:4, :], ohu[:, :, 0:2, 1:], ab2b2)
        nc.vector.tensor_tensor(ext[:], ext[:], exr[:], op=OP.add)
        nc.vector.tensor_tensor(sm[:], sm[:], ext[:], op=OP.add)
        nc.sync.dma_start(out_d[:, :, :, t0:t0 + TCH], sm[:])

    prev = None
    for c in range(NCH):
        d = prep(c)
        if prev is not None:
            epilogue(c - 1, prev)
        loop(c, d)
        prev = d
    epilogue(NCH - 1, prev)


_NC_CACHE = [None]


def kernel(params, rewards, choices, pids):
    """Full-input host wrapper: shard B across 8 cores, run SPMD, gather."""
    if _NC_CACHE[0] is None:
        _NC_CACHE[0] = build_nc()
    nc = _NC_CACHE[0]
    paramsT = np.ascontiguousarray(params.T, dtype=np.float32)   # [P, 13]
    in_maps = []
    for k in range(NCORE):
        s0 = k * BL
        in_maps.append({
            "rewards": np.ascontiguousarray(rewards[:, s0:s0 + BL].T, np.float32),
            "choices": np.ascontiguousarray(choices[:, s0:s0 + BL].T, np.int32),
            "paramsT": paramsT,
            "pids": np.ascontiguousarray(pids[s0:s0 + BL].reshape(G, 128).T, np.int32),
        })
    res = run_bass_kernel_spmd(nc, in_maps, list(range(NCORE)), trace=False)
    out = np.empty((T, B, A), np.float32)
    for k in range(NCORE):
        o = res.results[k]["out"]          # [128, G, A, T]
        out[:, k * BL:(k + 1) * BL, :] = o.transpose(3, 1, 0, 2).reshape(T, BL, A)
    return out

